# revision 1
# baseline (speedup 1.0000x reference)
"""Trainium2 Bass kernel v2 for the LSTM caption decoder.

Architecture (vs v1 baseline): the gate matmuls are FLIPPED — weights are the
stationary operand [K=128, M=128 gate units] and the hidden state streams as
the moving operand in transposed layout [K, batch=64].  TimelineSim charges
out_free x cycles_per_row, so the per-step gate cost drops from 32768 cycles
(h-stationary, M=64 wastes half the array) to 16384 in bf16 — and to 6144
with fp8e4 DoubleRow (2 K-tiles per instruction at 0.5 cyc/row).

Consequences of the flip:
  - gates emerge in [gate_unit, batch] = transposed layout; the whole
    elementwise c/h chain runs in [hidden, batch]; h_new IS the next step's
    moving operand. No per-step PE transposes of h.
  - the x-projection is FUSED into the gate matmul as 4 extra contraction
    chunks (W_ih columns): no token_proj phase, no [64,4096] gather; instead a
    small per-step embedding row gather [64,512] + 4 PE transposes.
  - logits keep h-stationary orientation (lhsT = h pair [128, 2x64],
    moving = lin_W.T in bf16) and run as PE gap filler, one vocab half per
    step. fp8 is NOT used for logits (accuracy).

Scaling for fp8e4 (ml_dtypes.float8_e4m3, max 240): W x 1024, x/h x 16;
psum is scaled 2^14, descaled in the ACT that applies the nonlinearity.
Numerics sim: rel err 9.0e-3 vs tolerance 2e-2.

Bias note: b_ih+b_hh from setup_inputs is always zero. If nonzero, an extra
stationary block + constant moving column adds the bias per gate unit.
"""

import sys

if "/opt/trn_rl_repo" not in sys.path:
    sys.path.insert(0, "/opt/trn_rl_repo")

import numpy as np
import ml_dtypes

import concourse.bass as bass
import concourse.mybir as mybir
import concourse.tile as tile
from concourse import bacc
from concourse.bass_utils import run_bass_kernel_spmd
from concourse.masks import make_identity

F32 = mybir.dt.float32
BF16 = mybir.dt.bfloat16
F8 = mybir.dt.float8e4
I32 = mybir.dt.int32
AF = mybir.ActivationFunctionType
ALU = mybir.AluOpType
DR = mybir.MatmulPerfMode.DoubleRow
NPF8 = ml_dtypes.float8_e4m3
NPBF = ml_dtypes.bfloat16

EMBED, HIDDEN, VOCAB = 512, 1024, 1004
B, T = 512, 65
NCORES = 8
BL = B // NCORES          # 64 batch rows per core
TS = T - 1                # 64 time steps
G4 = 4 * HIDDEN           # 4096 gate width
NM = G4 // 128            # 32 stationary m-tiles
NKI = EMBED // 128        # 4 ih k-chunks -> 2 dk pairs
NKH = HIDDEN // 128       # 8 hh k-chunks -> 4 dk pairs
NDK = (NKI + NKH) // 2    # 6 dk pairs per m-tile
VH = VOCAB // 2           # 502 vocab half

SW = 1024.0               # weight scale for fp8
SX = 16.0                 # x/h scale for fp8
DESCALE = 1.0 / (SW * SX)

# m-tile issue order: gate stream order g, i, o, f (torch gate index 2,0,3,1)
# f is streamed LAST: it heads the serial c->tanh->h chain, and putting it
# last lets the next step's ih matmuls for g/i/o start with no psum WAR wait.
GSTREAM = [2, 0, 3, 1]
MORDER = [g * 8 + j for g in GSTREAM for j in range(8)]

WCOLS = NM * NDK * 256    # 49152 fp8 cols of the stationary blob


def build_program(steps=TS, with_bias=False, with_linb=False):
    nc = bacc.Bacc("TRN2", target_bir_lowering=False, debug=False)

    wblob = nc.dram_tensor("wblob", [128, WCOLS], F8, kind="ExternalInput")
    embbf = nc.dram_tensor("embbf", [VOCAB, EMBED], BF16, kind="ExternalInput")
    linwt = nc.dram_tensor("linwt", [128, NKH * VOCAB], BF16,
                           kind="ExternalInput")
    linb = nc.dram_tensor("linb", [1, VOCAB], BF16, kind="ExternalInput")
    featbf = nc.dram_tensor("featbf", [128, 512], BF16, kind="ExternalInput")
    featsx = nc.dram_tensor("featsx", [128, 512], F32, kind="ExternalInput")
    h0f8 = nc.dram_tensor("h0f8", [128, 512], F8, kind="ExternalInput")
    caps = nc.dram_tensor("caps", [BL, TS], I32, kind="ExternalInput")
    if with_bias:
        bblk = nc.dram_tensor("bblk", [128, NM * 256], F8, kind="ExternalInput")
    else:
        bblk = None
    outd = nc.dram_tensor("out", [BL, TS, VOCAB], F32, kind="ExternalOutput")

    with tile.TileContext(nc) as tc:
        _body(nc, tc, steps, with_bias, with_linb, wblob.ap(), embbf.ap(),
              linwt.ap(), linb.ap(), featbf.ap(), featsx.ap(), h0f8.ap(),
              caps.ap(), bblk.ap() if bblk is not None else None, outd.ap())
    nc.compile()
    return nc


def _body(nc, tc, steps, with_bias, with_linb, wblob, embbf, linwt, linb,
          featbf, featsx, h0f8, caps, bblk, outd):
    with (
        tc.tile_pool(name="pg", bufs=1) as pg,
        tc.tile_pool(name="pb", bufs=1) as pb,
        tc.tile_pool(name="pp", bufs=1, space="PSUM") as pp,
    ):
        # ---------------- startup loads ----------------
        # small, early-needed tensors first: caps gates the first embedding
        # gather; feat/h0 gate the first h-dependent matmuls
        caps_sb = pg.tile([BL, TS], I32, tag="cap")
        nc.sync.dma_start(caps_sb[:], caps)
        ident = pg.tile([BL, BL], BF16, tag="id")
        make_identity(nc, ident[:])
        hf8_prev = pb.tile([128, 512], F8, tag="hf8", bufs=2, name="hf8_init")
        nc.sync.dma_start(hf8_prev[:], h0f8)
        c_prev = pb.tile([128, 512], BF16, tag="c", bufs=2, name="c_init")
        nc.vector.memset(c_prev[:], 0.0)
        featbf_sb = pg.tile([128, 512], BF16, tag="fb")
        nc.sync.dma_start(featbf_sb[:], featbf)
        featsx_sb = pg.tile([128, 512], F32, tag="fs")
        nc.sync.dma_start(featsx_sb[:], featsx)
        w_sb = pg.tile([128, WCOLS], F8, tag="w")
        # weight blocks in gate processing order (f, g, i, o) so the first
        # gate groups can start before the whole blob lands
        for g in (1, 2, 0, 3):
            c0, c1 = g * 8 * 1536, (g + 1) * 8 * 1536
            nc.sync.dma_start(w_sb[:, c0:c1], wblob[:, c0:c1])
        linwt_sb = pg.tile([128, NKH * VOCAB], BF16, tag="lw")
        nc.sync.dma_start(linwt_sb[:], linwt)
        if with_linb:
            linbbf_sb = pg.tile([1, VOCAB], BF16, tag="lb")
            nc.sync.dma_start(linbbf_sb[:], linb)
            onesrow_sb = pg.tile([1, 128], BF16, tag="ones1")
            nc.vector.memset(onesrow_sb[:], 1.0)
        if with_bias:
            bb_sb = pg.tile([128, NM * 256], F8, tag="bb")
            nc.sync.dma_start(bb_sb[:], bblk)
            ones_sb = pg.tile([128, 256], F8, tag="ones")
            nc.vector.memset(ones_sb[:], 0.0)
            nc.vector.memset(ones_sb[0:1, 0:64], SX)

        # ---------------- helpers ----------------
        def gather(t):
            e = pb.tile([BL, EMBED], BF16, tag="emb", bufs=4, name=f"emb_{t}")
            nc.gpsimd.indirect_dma_start(
                out=e[:], out_offset=None, in_=embbf,
                in_offset=bass.IndirectOffsetOnAxis(
                    ap=caps_sb[:, t:t + 1], axis=0))
            return e

        def transp(t, emb_t):
            ps = pp.tile([128, 256], BF16, tag="et", bufs=2, name=f"etp_{t}")
            for cchunk in range(4):
                nc.tensor.matmul(
                    ps[:, 64 * cchunk:64 * (cchunk + 1)],
                    lhsT=emb_t[:, 128 * cchunk:128 * (cchunk + 1)],
                    rhs=ident[:], is_transpose=True)
            ef = pb.tile([128, 256], F8, tag="ef8", bufs=2, name=f"ef8_{t}")
            nc.vector.tensor_scalar_mul(ef[:], ps[:], SX)
            return ef

        def dr_mm(out_ap, wcol, rhs_ap, start, stop):
            nc.tensor.matmul(
                out_ap,
                lhsT=w_sb[:, wcol:wcol + 256].rearrange(
                    "p (two m) -> p two m", two=2),
                rhs=rhs_ap, start=start, stop=stop, perf_mode=DR,
                skip_group_check=True)

        # pre-loop: embeddings for steps 0 and 1
        emb_t = {0: gather(0)}
        if steps > 1:
            emb_t[1] = gather(1)
        ef8_t = {0: transp(0, emb_t[0])}

        hp_t = {}         # hpair tiles by pair index

        def logits_half(t):
            # pair p halves run at iterations 2p+3 / 2p+4 — both at least
            # two steps after hp(p) completes, so the PE never waits on it
            p, nh = (t - 3) // 2, (t - 3) % 2
            if p > (steps - 2) // 2:
                return
            # separate psum tile per vocab half (own bank, no false deps)
            lp = pp.tile([128, 512], F32, tag=f"lps{nh}", bufs=1,
                         name=f"lp{nh}_{p}")
            lp_pairs.setdefault(p, [None, None])[nh] = lp
            hp = hp_t[p]
            s0 = nh * VH          # vocab start in lin_W
            for k in range(NKH):
                # hp layout [128, (chunk 8, step 2, batch 64)] — chunk-major
                # so the stationary lhsT [128, 128] is contiguous
                lhsT = hp[:, 128 * k:128 * (k + 1)]
                nc.tensor.matmul(
                    lp[:, 0:VH], lhsT=lhsT,
                    rhs=linwt_sb[:, k * VOCAB + s0:k * VOCAB + s0 + VH],
                    start=(k == 0), stop=(k == NKH - 1) and not with_linb,
                    skip_group_check=True)
            if with_linb:
                # fold lin_b in as a K=1 ones-row matmul (broadcast add)
                nc.tensor.matmul(
                    lp[:, 0:VH], lhsT=onesrow_sb[:],
                    rhs=linbbf_sb[:, s0:s0 + VH],
                    start=False, stop=True, skip_group_check=True)
            if nh == 1:
                del hp_t[p]

        lp_pairs = {}   # pair -> [lp0, lp1] psum tiles awaiting copy-out

        # gate processing order: f first (it heads the serial
        # c -> tanh -> h chain), o last (needed latest by the chain)
        GORDER = [1, 2, 0, 3]   # torch gate indices f, g, i, o

        # ---------------- main loop ----------------
        for t in range(steps):
            # Pool: embedding gather two steps ahead (Pool does only DMA)
            if t + 2 < steps:
                emb_t[t + 2] = gather(t + 2)

            # PE: transposes for t+1 (DVE converts to fp8)
            if t + 1 < steps:
                ef8_t[t + 1] = transp(t + 1, emb_t.pop(t + 1))

            # per-gate psum tiles (one bank each); a gate's matmul group for
            # step t starts long after its step t-1 ACT read, so bufs=1
            # carries no WAR stall
            gt = {g: pp.tile([128, 512], F32, tag=f"gp{g}", bufs=1,
                             name=f"gp{g}_{t}") for g in GORDER}
            ef8 = ef8_t.pop(t)

            # PE: logits gap filler (one vocab half per step)
            if t >= 3:
                logits_half(t)

            # PE: gate matmul groups, gate-major, split into m-halves so
            # the ACT reads pipeline with the matmuls.  Within an m-half:
            # dk0..3 (ih + first h half) for all its m-tiles, then dk4..5.
            for g in GORDER:
                for mh in range(2):
                    for j in range(4 * mh, 4 * mh + 4):
                        m = g * 8 + j
                        reg = j * 64
                        out_ap = gt[g][:, reg:reg + 64]
                        for dk in (0, 1):
                            # exactly ONE start per psum bank per step: a
                            # start poisons the whole 2KB bank as pending-
                            # zero, which auto-zeroes every region's first
                            # write; a second start would clobber siblings
                            dr_mm(out_ap, (m * NDK + dk) * 256,
                                  ef8[:, 128 * dk:128 * (dk + 1)].rearrange(
                                      "p (two n) -> p two n", two=2),
                                  start=(mh == 0 and j == 0 and dk == 0),
                                  stop=False)
                        if with_bias:
                            nc.tensor.matmul(
                                out_ap,
                                lhsT=bb_sb[:, m * 256:m * 256 + 256]
                                .rearrange("p (two m2) -> p two m2", two=2),
                                rhs=ones_sb[:].rearrange(
                                    "p (two n) -> p two n", two=2),
                                start=False, stop=False, perf_mode=DR,
                                skip_group_check=True)
                        for dk in (2, 3):
                            dr_mm(out_ap, (m * NDK + dk) * 256,
                                  hf8_prev[:, 128 * (dk - 2):128 * (dk - 1)]
                                  .rearrange("p (two n) -> p two n", two=2),
                                  start=False, stop=False)
                    for j in range(4 * mh, 4 * mh + 4):
                        m = g * 8 + j
                        reg = j * 64
                        for dk in (4, 5):
                            dr_mm(gt[g][:, reg:reg + 64],
                                  (m * NDK + dk) * 256,
                                  hf8_prev[:, 128 * (dk - 2):128 * (dk - 1)]
                                  .rearrange("p (two n) -> p two n", two=2),
                                  start=False, stop=(dk == 5))

            # ACT: gate nonlinearities, full tiles (the serial ff->gg->ii
            # sequence gates the c chain; fewer, larger legs finish sooner)
            ff = pb.tile([128, 512], BF16, tag="ff", bufs=1, name=f"ff_{t}")
            gg = pb.tile([128, 512], BF16, tag="gg", bufs=1, name=f"gg_{t}")
            ii = pb.tile([128, 512], BF16, tag="ii", bufs=1, name=f"ii_{t}")
            oo = pb.tile([128, 512], BF16, tag="oo", bufs=1, name=f"oo_{t}")
            for dst, g, fn in ((ff, 1, AF.Sigmoid), (gg, 2, AF.Tanh),
                               (ii, 0, AF.Sigmoid), (oo, 3, AF.Sigmoid)):
                nc.scalar.activation(dst[:], gt[g][:], fn, scale=DESCALE)

            # Pool: t1 = f * c_prev (off the critical DVE/ACT engines)
            t1 = pb.tile([128, 512], BF16, tag="t1", bufs=1, name=f"t1_{t}")
            for q in range(2):
                sl = slice(256 * q, 256 * (q + 1))
                nc.gpsimd.tensor_mul(t1[:, sl], ff[:, sl], c_prev[:, sl])

            # DVE: t2 + c; ACT: tanh(c) — all in halves
            t2 = pb.tile([128, 512], BF16, tag="t2", bufs=1, name=f"t2_{t}")
            c_new = pb.tile([128, 512], BF16, tag="c", bufs=2, name=f"c_{t}")
            tc_h = pb.tile([128, 512], BF16, tag="tc", bufs=1, name=f"tc_{t}")
            for q in range(2):
                sl = slice(256 * q, 256 * (q + 1))
                nc.vector.tensor_mul(t2[:, sl], ii[:, sl], gg[:, sl])
                nc.vector.tensor_add(c_new[:, sl], t1[:, sl], t2[:, sl])
                nc.scalar.activation(tc_h[:, sl], c_new[:, sl], AF.Tanh)

            # tail in halves: t3 = o*tanh(c), hf8 (DVE); bf16 h for the
            # logits on Pool (SBUF-only operands)
            if t % 2 == 0:
                hp = pb.tile([128, 1024], BF16, tag="hp", bufs=3,
                             name=f"hp_{t // 2}")
                hp_t[t // 2] = hp
            else:
                hp = hp_t[t // 2]
            hf8 = pb.tile([128, 512], F8, tag="hf8", bufs=2, name=f"hf8_{t}")
            t3 = pb.tile([128, 512], BF16, tag="t3", bufs=1, name=f"t3_{t}")
            hp4 = hp[:].rearrange("p (cc s b) -> p cc s b", cc=8, s=2)
            for q in range(2):
                sl = slice(256 * q, 256 * (q + 1))
                nc.vector.tensor_mul(t3[:, sl], oo[:, sl], tc_h[:, sl])
                nc.vector.scalar_tensor_tensor(
                    out=hf8[:, sl], in0=t3[:, sl], scalar=SX,
                    in1=featsx_sb[:, sl], op0=ALU.mult, op1=ALU.add)
            for q in range(2):
                sl = slice(256 * q, 256 * (q + 1))
                nc.vector.tensor_add(
                    hp4[:, 4 * q:4 * (q + 1), t % 2, :],
                    t3[:, sl].rearrange("p (cc b) -> p cc b", cc=4),
                    featbf_sb[:, sl].rearrange("p (cc b) -> p cc b", cc=4))
            hf8_prev = hf8
            c_prev = c_new

        # ---------------- drain remaining logits ----------------
        for t in range(steps, steps + 4):
            if t >= 3:
                logits_half(t)

        # Deferred psum -> SBUF copies + output DMAs, emitted last so the
        # greedy scheduler places them only in genuinely idle ACT slots
        # (they must still land before the next-but-one pair reuses the
        # psum bank, which the scheduler's WAR handling enforces).
        for p in sorted(lp_pairs):
            lp0, lp1 = lp_pairs[p]
            ls = pb.tile([128, VOCAB], F32, tag="ls", bufs=2, name=f"ls_{p}")
            # quarter-sized copies cap how long a copy can occupy ACT when
            # the scheduler slots one just before a chain leg becomes ready
            for q in range(2):
                qs = slice(q * 251, (q + 1) * 251)
                nc.scalar.activation(ls[:, q * 251:(q + 1) * 251],
                                     lp0[:, qs], AF.Copy)
            for q in range(2):
                nc.scalar.activation(
                    ls[:, VH + q * 251:VH + (q + 1) * 251],
                    lp1[:, q * 251:(q + 1) * 251], AF.Copy)
            nc.sync.dma_start(outd[:, 2 * p, :], ls[0:BL])
            nc.sync.dma_start(outd[:, 2 * p + 1, :], ls[BL:128])


# ---------------------------------------------------------------------------
# host glue
# ---------------------------------------------------------------------------

_CACHE = {}


def _get_program(steps=TS, with_bias=False, with_linb=False):
    key = (steps, with_bias, with_linb)
    if key not in _CACHE:
        _CACHE[key] = build_program(steps, with_bias, with_linb)
    return _CACHE[key]


def make_in_maps(feature, captions, embed_W, W_ih, W_hh, b_ih, b_hh,
                 lin_W, lin_b, steps=TS):
    f32 = np.float32
    bvec = (np.asarray(b_ih) + np.asarray(b_hh)).astype(f32)
    with_bias = bool(np.any(bvec != 0.0))
    with_linb = bool(np.any(np.asarray(lin_b) != 0.0))

    # stationary fp8 blob: block (m, dk, i) = W_all.T chunk
    W_all = np.concatenate([W_ih.astype(f32), W_hh.astype(f32)], axis=1)
    WT = np.ascontiguousarray(W_all.T) * SW           # [1536, 4096]
    arr = WT.reshape(NDK, 2, 128, NM, 128)            # [dk, i, p, m, ml]
    wblob = np.ascontiguousarray(
        arr.transpose(2, 3, 0, 1, 4).reshape(128, WCOLS)).astype(NPF8)

    linwt_p = np.ascontiguousarray(
        lin_W.astype(f32).T.reshape(NKH, 128, VOCAB)
        .transpose(1, 0, 2).reshape(128, NKH * VOCAB)).astype(NPBF)

    shared = {
        "wblob": wblob,
        "embbf": np.ascontiguousarray(embed_W.astype(f32)).astype(NPBF),
        "linwt": linwt_p,
        "linb": lin_b.astype(f32).reshape(1, VOCAB).astype(NPBF),
    }
    if with_bias:
        # block m: [p, i, ml]; only (p=0, i=0) row nonzero = b[gate]*SW
        bb = np.zeros((128, NM, 2, 128), dtype=f32)
        bb[0, :, 0, :] = (bvec * SW).reshape(NM, 128)
        shared["bblk"] = np.ascontiguousarray(
            bb.reshape(128, NM * 256)).astype(NPF8)

    in_maps = []
    for i in range(NCORES):
        sl = slice(i * BL, (i + 1) * BL)
        fl = feature[sl].astype(f32)                  # [64, 1024]
        featT = np.ascontiguousarray(
            fl.T.reshape(NKH, 128, BL).transpose(1, 0, 2).reshape(128, 512))
        m = dict(shared)
        m["featbf"] = featT.astype(NPBF)
        m["featsx"] = np.ascontiguousarray(featT * SX)
        m["h0f8"] = (featT * SX).astype(NPF8)
        cp = np.zeros((BL, TS), np.int32)
        cp[:, :steps] = captions[sl, :steps].astype(np.int32)
        m["caps"] = cp
        in_maps.append(m)
    return in_maps, with_bias, with_linb


def kernel(feature, captions, lengths=None, embed_W=None, W_ih=None,
           W_hh=None, b_ih=None, b_hh=None, lin_W=None, lin_b=None,
           trace=False, steps=TS):
    feature = np.asarray(feature)
    captions = np.asarray(captions)
    in_maps, with_bias, with_linb = make_in_maps(
        feature, captions, np.asarray(embed_W), np.asarray(W_ih),
        np.asarray(W_hh), np.asarray(b_ih), np.asarray(b_hh),
        np.asarray(lin_W), np.asarray(lin_b), steps=steps)
    nc = _get_program(steps, with_bias, with_linb)
    res = run_bass_kernel_spmd(nc, in_maps, list(range(NCORES)), trace=trace)
    outp = np.concatenate([res.results[i]["out"] for i in range(NCORES)],
                          axis=0)
    if trace:
        kernel.last_exec_time_ns = res.exec_time_ns
        kernel.last_results = res
    return outp

